# revision 23
# baseline (speedup 1.0000x reference)
"""Causal self-attention (B=2, S=2048, D=1024, H=16, Hd=64) on 8 TRN2 NeuronCores.

Sharding: tensor-parallel over heads (4 heads/core) x data-parallel over batch
(cores 0-3 -> batch 0, cores 4-7 -> batch 1). Each core:
  - computes q^T,k^T (transposed layout, heads stacked in pairs on partitions)
    and v (natural layout) for its 4 heads
  - runs causal attention in transposed-score layout (scores_T[k, q]) so no
    transposes are ever needed; softmax denominators come for free from a
    ones-column appended to V; normalization via DVE reciprocal + gpsimd
    partition_broadcast
  - computes its partial output projection y_part = out_heads @ W_proj[rows]
Host sums the 4 partials per batch and adds b_proj (the unshard step for a
row-parallel matmul). Matmul datapath is bf16 (fp32 PSUM accumulation).
"""

import sys

if "/opt/trn_rl_repo" not in sys.path:
    sys.path.insert(0, "/opt/trn_rl_repo")

import ml_dtypes
import numpy as np

D = 1024
S = 2048
B = 2
H = 16
HD = 64
N_CORES = 8
GROUPS = 4  # cores per batch
HPC = 4  # heads per core
SCALE = 1.0 / np.sqrt(HD)
NEG = -1.0e9

_module_cache = {}


def _build_module():
    if "nc" in _module_cache:
        return _module_cache["nc"]

    import concourse.bacc as bacc
    import concourse.mybir as mybir
    import concourse.tile as tile
    from concourse.bass import ts

    f32 = mybir.dt.float32
    bf16 = mybir.dt.bfloat16
    AF = mybir.ActivationFunctionType

    nc = bacc.Bacc("TRN2", target_bir_lowering=False, debug=False)

    xT = nc.dram_tensor("xT", [D, S], bf16, kind="ExternalInput")
    w_qk = nc.dram_tensor("w_qk", [D, 512], bf16, kind="ExternalInput")
    b_qk = nc.dram_tensor("b_qk", [128, 4], f32, kind="ExternalInput")
    w_v = nc.dram_tensor("w_v", [D, 256], bf16, kind="ExternalInput")
    b_v = nc.dram_tensor("b_v", [128, 256], f32, kind="ExternalInput")
    w_pr = nc.dram_tensor("w_pr", [256, D], bf16, kind="ExternalInput")
    mask = nc.dram_tensor("mask", [128, 128], f32, kind="ExternalInput")
    y = nc.dram_tensor("y", [S, D], f32, kind="ExternalOutput")

    KT = D // 128  # 8 contraction tiles
    ST = S // 128  # 16 seq tiles

    import contextlib

    with tile.TileContext(nc) as tc:
        with contextlib.ExitStack() as ctx2:
            const = ctx2.enter_context(tc.tile_pool(name="const", bufs=1))
            psA = ctx2.enter_context(tc.tile_pool(name="psA", bufs=1, space="PSUM"))
            # ---- resident SBUF tensors ----
            xT_sb = const.tile([128, KT, S], bf16)
            wqk_sb = const.tile([128, KT, 512], bf16)
            wv_sb = const.tile([128, KT, 256], bf16)
            bqk_sb = const.tile([128, 4], f32)
            bv_sb = const.tile([128, 256], f32)
            wpr_sb = const.tile([128, 2, D], bf16)
            mask_sb = const.tile([128, 128], f32)
            ones_sb = const.tile([1, 64], f32)
            warm_sb = const.tile([1, 64], f32)
            qkT_sb = const.tile([128, 4, S], bf16)  # m: q01,q23,k01,k23
            oU_sb = const.tile([65, 4, S], bf16)  # unnormalized attn out + sums
            v_sb = const.tile([128, ST, 4 * 65], bf16)  # 4 heads x 65 (ones col)
            oT_sb = const.tile([128, 2, S], bf16)  # normalized attn out, pair-stacked

            for k in range(KT):
                nc.sync.dma_start(out=xT_sb[:, k, :], in_=xT[ts(k, 128), :])
                nc.sync.dma_start(out=wqk_sb[:, k, :], in_=w_qk[ts(k, 128), :])
                nc.sync.dma_start(out=wv_sb[:, k, :], in_=w_v[ts(k, 128), :])
            nc.sync.dma_start(out=bqk_sb[:], in_=b_qk[:])
            nc.sync.dma_start(out=bv_sb[:], in_=b_v[:])
            nc.sync.dma_start(out=wpr_sb[:, 0, :], in_=w_pr[0:128, :])
            nc.sync.dma_start(out=wpr_sb[:, 1, :], in_=w_pr[128:256, :])
            nc.sync.dma_start(out=mask_sb[:], in_=mask[:])
            nc.vector.memset(ones_sb[:], 1.0)
            # preload the ACT exp table set early, off the critical path
            nc.scalar.activation(warm_sb[:], ones_sb[:], AF.Exp)
            for h in range(HPC):
                nc.vector.memset(v_sb[:, :, h * 65 + 64 : h * 65 + 65], 1.0)

            # ---- phase B helper: one q^T/k^T M-tile, accumulated over k ----
            def qk_tile(m):
                for half in range(2):
                    ps = psA.tile([128, 1024], f32, tag="big", name="ps_qk")
                    for k in range(KT):
                        for j in range(2):
                            nc.tensor.matmul(
                                ps[:, ts(j, 512)],
                                lhsT=wqk_sb[:, k, ts(m, 128)],
                                rhs=xT_sb[:, k, half * 1024 + j * 512 : half * 1024 + j * 512 + 512],
                                start=(k == 0),
                                stop=(k == KT - 1),
                            )
                    # evict with per-partition bias (q scale folded into W)
                    nc.scalar.activation(
                        qkT_sb[:, m, ts(half, 1024)],
                        ps[:],
                        AF.Identity,
                        bias=bqk_sb[:, m : m + 1],
                    )

            def v_quarter(qt):
                ps = psA.tile([128, 1024], f32, tag="big", name="ps_v")
                for sti in range(4):
                    st = qt * 4 + sti
                    for k in range(KT):
                        nc.tensor.matmul(
                            ps[:, ts(sti, 256)],
                            lhsT=xT_sb[:, k, ts(st, 128)],
                            rhs=wv_sb[:, k, :],
                            start=(k == 0),
                            stop=(k == KT - 1),
                        )
                    for h in range(HPC):
                        nc.vector.tensor_add(
                            v_sb[:, st, h * 65 : h * 65 + 64],
                            ps[:, sti * 256 + h * 64 : sti * 256 + h * 64 + 64],
                            bv_sb[:, ts(h, 64)],
                        )

            # pair-0 inputs first so attention can start early
            qk_tile(0)
            qk_tile(2)
            for qt in range(4):
                v_quarter(qt)

            # ---- phase D: attention, pair-packed; phase E: projection ----
            # PSUM budget: psA 1x2 + psS 2x1 + psO 2tags x 2banks = 8 banks.
            ptp = ctx2.enter_context(tc.tile_pool(name="pt", bufs=3))
            ysbp = ctx2.enter_context(tc.tile_pool(name="ysb", bufs=3))
            nrm = ctx2.enter_context(tc.tile_pool(name="nrm", bufs=2))
            psS = ctx2.enter_context(tc.tile_pool(name="psS", bufs=2, space="PSUM"))
            psO = ctx2.enter_context(tc.tile_pool(name="psO", bufs=1, space="PSUM"))
            for hp in range(2):
                if hp == 1:
                    qk_tile(1)
                    qk_tile(3)
                for jj in range(2):
                    c0 = 1024 * jj
                    n_sk = 8 * jj + 8
                    po = [
                        psO.tile([65, 1024], f32, tag=f"o{h}", name=f"po{h}")
                        for h in range(2)
                    ]
                    for sk in range(n_sk):
                        col0 = max(c0, sk * 128)
                        rel0 = col0 - c0
                        # po-relative sub-chunks, split at the 512 bank line
                        subs = []
                        if rel0 < 512:
                            subs.append((rel0, 512))
                        subs.append((max(rel0, 512), 1024))
                        for h in range(2):
                            hr = h * 64
                            pt = ptp.tile(
                                [128, 1024], bf16, tag=f"pt{h}", name=f"pt{h}"
                            )
                            pss = []
                            for s0, s1 in subs:
                                ps = psS.tile(
                                    [128, 512], f32, tag="s", name=f"ps{h}"
                                )
                                pss.append(ps)
                                nc.tensor.matmul(
                                    ps[:, : s1 - s0],
                                    lhsT=qkT_sb[hr : hr + 64, 2 + hp, ts(sk, 128)],
                                    rhs=qkT_sb[hr : hr + 64, hp, c0 + s0 : c0 + s1],
                                    start=True,
                                    stop=True,
                                    tile_position=(hr, 0),
                                )
                            if col0 == sk * 128:
                                nc.vector.tensor_add(
                                    pss[0][:, 0:128], pss[0][:, 0:128], mask_sb[:]
                                )
                            for (s0, s1), ps in zip(subs, pss):
                                nc.scalar.activation(
                                    pt[:, s0:s1], ps[:, : s1 - s0], AF.Exp
                                )
                            sk_b0 = 8 * jj + 3  # last sk writing bank 0
                            for s0, s1 in subs:
                                last = (sk == sk_b0) if s1 <= 512 else (sk == n_sk - 1)
                                nc.tensor.matmul(
                                    po[h][:, s0:s1],
                                    lhsT=v_sb[
                                        :,
                                        sk,
                                        (2 * hp + h) * 65 : (2 * hp + h) * 65 + 65,
                                    ],
                                    rhs=pt[:, s0:s1],
                                    start=(sk == 0),
                                    stop=last,
                                )
                    # fast eviction (normalization deferred off the hot loop)
                    for h in range(2):
                        nc.vector.tensor_copy(
                            oU_sb[:, 2 * hp + h, c0 : c0 + 1024], po[h][:]
                        )
                # bulk normalization for this pair
                for h in range(2):
                    hr = h * 64
                    hi = 2 * hp + h
                    rec = nrm.tile([1, S], f32, tag="rec")
                    nc.vector.reciprocal(rec[:], oU_sb[64:65, hi, :])
                    rbc = nrm.tile([64, S], f32, tag="rbc")
                    nc.gpsimd.partition_broadcast(rbc[:], rec[:])
                    nc.vector.tensor_mul(
                        oT_sb[hr : hr + 64, hp, :],
                        oU_sb[0:64, hi, :],
                        rbc[:],
                    )

            # ---- phase E: y_part[m*128:, :] = sum_kp oT[kp]^T @ w_pr[kp] ----
            for m in range(ST):
                y_sb = ysbp.tile([128, 1024], f32, tag="ysb")
                for nch in range(2):
                    ps = psS.tile([128, 512], f32, tag="s", name="psy")
                    for kp in range(2):
                        nc.tensor.matmul(
                            ps[:],
                            lhsT=oT_sb[:, kp, ts(m, 128)],
                            rhs=wpr_sb[:, kp, ts(nch, 512)],
                            start=(kp == 0),
                            stop=(kp == 1),
                        )
                    nc.scalar.copy(y_sb[:, ts(nch, 512)], ps[:])
                nc.sync.dma_start(out=y[ts(m, 128), :], in_=y_sb[:])

    nc.finalize()
    _module_cache["nc"] = nc
    return nc


def _shard_inputs(x, w_qkv, b_qkv, w_proj):
    """Per-core input dicts. Core c: batch c//4, heads 4*(c%4) .. 4*(c%4)+3."""
    bf = ml_dtypes.bfloat16
    in_maps = []
    mask_np = np.where(
        np.arange(128)[:, None] <= np.arange(128)[None, :], 0.0, NEG
    ).astype(np.float32)
    xTs = [np.ascontiguousarray(x[b].T).astype(bf) for b in range(B)]
    for c in range(N_CORES):
        b = c // GROUPS
        g = c % GROUPS
        qc = slice(256 * g, 256 * g + 256)
        kc = slice(D + 256 * g, D + 256 * g + 256)
        vc = slice(2 * D + 256 * g, 2 * D + 256 * g + 256)
        # 1/sqrt(hd) scale folded into the q columns of W and into b_q
        w_qk = np.ascontiguousarray(
            np.concatenate([w_qkv[:, qc] * SCALE, w_qkv[:, kc]], axis=1)
        ).astype(bf)
        bq = np.concatenate([b_qkv[qc] * SCALE, b_qkv[kc]]).astype(np.float32)
        b_qk = np.ascontiguousarray(bq.reshape(4, 128).T)
        w_v = np.ascontiguousarray(w_qkv[:, vc]).astype(bf)
        b_v = np.ascontiguousarray(np.broadcast_to(b_qkv[vc], (128, 256))).astype(
            np.float32
        )
        w_pr = np.ascontiguousarray(w_proj[256 * g : 256 * g + 256, :]).astype(bf)
        in_maps.append(
            {
                "xT": xTs[b],
                "w_qk": w_qk,
                "b_qk": b_qk,
                "w_v": w_v,
                "b_v": b_v,
                "w_pr": w_pr,
                "mask": mask_np,
            }
        )
    return in_maps


def kernel(x, w_qkv, b_qkv, w_proj, b_proj, _spmd_kwargs=None):
    from concourse.bass_utils import run_bass_kernel_spmd

    x = np.asarray(x, dtype=np.float32)
    w_qkv = np.asarray(w_qkv, dtype=np.float32)
    b_qkv = np.asarray(b_qkv, dtype=np.float32)
    w_proj = np.asarray(w_proj, dtype=np.float32)
    b_proj = np.asarray(b_proj, dtype=np.float32)

    nc = _build_module()
    in_maps = _shard_inputs(x, w_qkv, b_qkv, w_proj)
    res = run_bass_kernel_spmd(
        nc, in_maps, list(range(N_CORES)), **(_spmd_kwargs or {})
    )
    out = np.empty((B, S, D), dtype=np.float32)
    for b in range(B):
        acc = np.zeros((S, D), dtype=np.float64)
        for g in range(GROUPS):
            acc += res.results[b * GROUPS + g]["y"]
        out[b] = (acc + b_proj.astype(np.float64)).astype(np.float32)
    if _spmd_kwargs:
        kernel.last_result = res
    return out


# revision 24
# speedup vs baseline: 1.1891x; 1.1891x over previous
"""Causal self-attention (B=2, S=2048, D=1024, H=16, Hd=64) on 8 TRN2 NeuronCores.

Sharding: tensor-parallel over heads (4 heads/core) x data-parallel over batch
(cores 0-3 -> batch 0, cores 4-7 -> batch 1). Each core:
  - computes q^T,k^T (transposed layout, heads stacked in pairs on partitions)
    and v (natural layout) for its 4 heads
  - runs causal attention in transposed-score layout (scores_T[k, q]) so no
    transposes are ever needed; softmax denominators come for free from a
    ones-column appended to V; normalization via DVE reciprocal + gpsimd
    partition_broadcast
  - computes its partial output projection y_part = out_heads @ W_proj[rows]
Host sums the 4 partials per batch and adds b_proj (the unshard step for a
row-parallel matmul). Matmul datapath is bf16 (fp32 PSUM accumulation).
"""

import sys

if "/opt/trn_rl_repo" not in sys.path:
    sys.path.insert(0, "/opt/trn_rl_repo")

import ml_dtypes
import numpy as np

D = 1024
S = 2048
B = 2
H = 16
HD = 64
N_CORES = 8
GROUPS = 4  # cores per batch
HPC = 4  # heads per core
SCALE = 1.0 / np.sqrt(HD)
NEG = -1.0e9

_module_cache = {}


def _build_module():
    if "nc" in _module_cache:
        return _module_cache["nc"]

    import concourse.bacc as bacc
    import concourse.mybir as mybir
    import concourse.tile as tile
    from concourse.bass import ts

    f32 = mybir.dt.float32
    bf16 = mybir.dt.bfloat16
    AF = mybir.ActivationFunctionType

    nc = bacc.Bacc("TRN2", target_bir_lowering=False, debug=False)

    xT = nc.dram_tensor("xT", [D, S], bf16, kind="ExternalInput")
    w_qk = nc.dram_tensor("w_qk", [D, 512], bf16, kind="ExternalInput")
    b_qk = nc.dram_tensor("b_qk", [128, 4], f32, kind="ExternalInput")
    w_v = nc.dram_tensor("w_v", [D, 256], bf16, kind="ExternalInput")
    b_v = nc.dram_tensor("b_v", [128, 256], f32, kind="ExternalInput")
    w_pr = nc.dram_tensor("w_pr", [256, D], bf16, kind="ExternalInput")
    mask = nc.dram_tensor("mask", [128, 128], f32, kind="ExternalInput")
    y = nc.dram_tensor("y", [S, D], f32, kind="ExternalOutput")

    KT = D // 128  # 8 contraction tiles
    ST = S // 128  # 16 seq tiles

    import contextlib

    with tile.TileContext(nc) as tc:
        with contextlib.ExitStack() as ctx2:
            const = ctx2.enter_context(tc.tile_pool(name="const", bufs=1))
            psA = ctx2.enter_context(tc.tile_pool(name="psA", bufs=1, space="PSUM"))
            # ---- resident SBUF tensors ----
            xT_sb = const.tile([128, KT, S], bf16)
            wqk_sb = const.tile([128, KT, 512], bf16)
            wv_sb = const.tile([128, KT, 256], bf16)
            bqk_sb = const.tile([128, 4], f32)
            bv_sb = const.tile([128, 256], f32)
            wpr_sb = const.tile([128, 2, D], bf16)
            mask_sb = const.tile([128, 128], f32)
            ones_sb = const.tile([1, 64], f32)
            warm_sb = const.tile([1, 64], f32)
            qkT_sb = const.tile([128, 4, S], bf16)  # m: q01,q23,k01,k23
            v_sb = const.tile([128, ST, 4 * 65], bf16)  # 4 heads x 65 (ones col)
            oT_sb = const.tile([128, 2, S], bf16)  # normalized attn out, pair-stacked

            for k in range(KT):
                nc.sync.dma_start(out=xT_sb[:, k, :], in_=xT[ts(k, 128), :])
                nc.sync.dma_start(out=wqk_sb[:, k, :], in_=w_qk[ts(k, 128), :])
                nc.sync.dma_start(out=wv_sb[:, k, :], in_=w_v[ts(k, 128), :])
            nc.sync.dma_start(out=bqk_sb[:], in_=b_qk[:])
            nc.sync.dma_start(out=bv_sb[:], in_=b_v[:])
            nc.sync.dma_start(out=wpr_sb[:, 0, :], in_=w_pr[0:128, :])
            nc.sync.dma_start(out=wpr_sb[:, 1, :], in_=w_pr[128:256, :])
            nc.sync.dma_start(out=mask_sb[:], in_=mask[:])
            nc.vector.memset(ones_sb[:], 1.0)
            # preload the ACT exp table set early, off the critical path
            nc.scalar.activation(warm_sb[:], ones_sb[:], AF.Exp)
            for h in range(HPC):
                nc.vector.memset(v_sb[:, :, h * 65 + 64 : h * 65 + 65], 1.0)

            # ---- phase B helper: one q^T/k^T M-tile, accumulated over k ----
            def qk_tile(m):
                for half in range(2):
                    ps = psA.tile([128, 1024], f32, tag="big", name="ps_qk")
                    for k in range(KT):
                        for j in range(2):
                            nc.tensor.matmul(
                                ps[:, ts(j, 512)],
                                lhsT=wqk_sb[:, k, ts(m, 128)],
                                rhs=xT_sb[:, k, half * 1024 + j * 512 : half * 1024 + j * 512 + 512],
                                start=(k == 0),
                                stop=(k == KT - 1),
                            )
                    # evict with per-partition bias (q scale folded into W)
                    nc.scalar.activation(
                        qkT_sb[:, m, ts(half, 1024)],
                        ps[:],
                        AF.Identity,
                        bias=bqk_sb[:, m : m + 1],
                    )

            def v_quarter(qt):
                ps = psA.tile([128, 1024], f32, tag="big", name="ps_v")
                for sti in range(4):
                    st = qt * 4 + sti
                    for k in range(KT):
                        nc.tensor.matmul(
                            ps[:, ts(sti, 256)],
                            lhsT=xT_sb[:, k, ts(st, 128)],
                            rhs=wv_sb[:, k, :],
                            start=(k == 0),
                            stop=(k == KT - 1),
                        )
                    for h in range(HPC):
                        nc.vector.tensor_add(
                            v_sb[:, st, h * 65 : h * 65 + 64],
                            ps[:, sti * 256 + h * 64 : sti * 256 + h * 64 + 64],
                            bv_sb[:, ts(h, 64)],
                        )

            # pair-0 inputs first so attention can start early
            qk_tile(0)
            qk_tile(2)
            for qt in range(4):
                v_quarter(qt)

            # ---- phase D: attention, pair-packed; phase E: projection ----
            # PSUM budget: psA 1x2 + psS 2x1 + psO 2tags x 2banks = 8 banks.
            ptp = ctx2.enter_context(tc.tile_pool(name="pt", bufs=3))
            ysbp = ctx2.enter_context(tc.tile_pool(name="ysb", bufs=3))
            nrm = ctx2.enter_context(tc.tile_pool(name="nrm", bufs=2))
            psS = ctx2.enter_context(tc.tile_pool(name="psS", bufs=2, space="PSUM"))
            psO = ctx2.enter_context(tc.tile_pool(name="psO", bufs=2, space="PSUM"))
            for hp in range(2):
                if hp == 1:
                    qk_tile(1)
                    qk_tile(3)
                for jj in range(4):
                    c0 = 512 * jj
                    n_sk = 4 * jj + 4
                    po = [
                        psO.tile([65, 512], f32, tag=f"o{h}", name=f"po{h}")
                        for h in range(2)
                    ]
                    for sk in range(n_sk):
                        col0 = max(c0, sk * 128)
                        n = c0 + 512 - col0
                        pss = []
                        pts = []
                        for h in range(2):
                            hr = h * 64
                            ps = psS.tile([128, 512], f32, tag="s", name=f"ps{h}")
                            pss.append(ps)
                            nc.tensor.matmul(
                                ps[:, :n],
                                lhsT=qkT_sb[hr : hr + 64, 2 + hp, ts(sk, 128)],
                                rhs=qkT_sb[hr : hr + 64, hp, col0 : col0 + n],
                                start=True,
                                stop=True,
                                tile_position=(hr, 0),
                            )
                        if col0 == sk * 128:
                            for h in range(2):
                                nc.vector.tensor_add(
                                    pss[h][:, 0:128], pss[h][:, 0:128], mask_sb[:]
                                )
                        for h in range(2):
                            pt = ptp.tile([128, 512], bf16, tag=f"pt{h}", name=f"pt{h}")
                            pts.append(pt)
                            nc.scalar.activation(pt[:, :n], pss[h][:, :n], AF.Exp)
                        for h in range(2):
                            nc.tensor.matmul(
                                po[h][:, col0 - c0 : col0 - c0 + n],
                                lhsT=v_sb[
                                    :, sk, (2 * hp + h) * 65 : (2 * hp + h) * 65 + 65
                                ],
                                rhs=pts[h][:, :n],
                                start=(sk == 0),
                                stop=(sk == n_sk - 1),
                            )
                    for h in range(2):
                        hr = h * 64
                        rec = nrm.tile([1, 512], f32, tag="rec")
                        nc.vector.reciprocal(rec[:], po[h][64:65, :])
                        rbc = nrm.tile([64, 512], f32, tag="rbc")
                        nc.gpsimd.partition_broadcast(rbc[:], rec[:])
                        nc.vector.tensor_mul(
                            oT_sb[hr : hr + 64, hp, c0 : c0 + 512],
                            po[h][0:64, :],
                            rbc[:],
                        )

            # ---- phase E: y_part[m*128:, :] = sum_kp oT[kp]^T @ w_pr[kp] ----
            for m in range(ST):
                y_sb = ysbp.tile([128, 1024], f32, tag="ysb")
                for nch in range(2):
                    ps = psS.tile([128, 512], f32, tag="s", name="psy")
                    for kp in range(2):
                        nc.tensor.matmul(
                            ps[:],
                            lhsT=oT_sb[:, kp, ts(m, 128)],
                            rhs=wpr_sb[:, kp, ts(nch, 512)],
                            start=(kp == 0),
                            stop=(kp == 1),
                        )
                    nc.scalar.copy(y_sb[:, ts(nch, 512)], ps[:])
                nc.sync.dma_start(out=y[ts(m, 128), :], in_=y_sb[:])

    nc.finalize()
    _module_cache["nc"] = nc
    return nc


def _shard_inputs(x, w_qkv, b_qkv, w_proj):
    """Per-core input dicts. Core c: batch c//4, heads 4*(c%4) .. 4*(c%4)+3."""
    bf = ml_dtypes.bfloat16
    in_maps = []
    mask_np = np.where(
        np.arange(128)[:, None] <= np.arange(128)[None, :], 0.0, NEG
    ).astype(np.float32)
    xTs = [np.ascontiguousarray(x[b].T).astype(bf) for b in range(B)]
    for c in range(N_CORES):
        b = c // GROUPS
        g = c % GROUPS
        qc = slice(256 * g, 256 * g + 256)
        kc = slice(D + 256 * g, D + 256 * g + 256)
        vc = slice(2 * D + 256 * g, 2 * D + 256 * g + 256)
        # 1/sqrt(hd) scale folded into the q columns of W and into b_q
        w_qk = np.ascontiguousarray(
            np.concatenate([w_qkv[:, qc] * SCALE, w_qkv[:, kc]], axis=1)
        ).astype(bf)
        bq = np.concatenate([b_qkv[qc] * SCALE, b_qkv[kc]]).astype(np.float32)
        b_qk = np.ascontiguousarray(bq.reshape(4, 128).T)
        w_v = np.ascontiguousarray(w_qkv[:, vc]).astype(bf)
        b_v = np.ascontiguousarray(np.broadcast_to(b_qkv[vc], (128, 256))).astype(
            np.float32
        )
        w_pr = np.ascontiguousarray(w_proj[256 * g : 256 * g + 256, :]).astype(bf)
        in_maps.append(
            {
                "xT": xTs[b],
                "w_qk": w_qk,
                "b_qk": b_qk,
                "w_v": w_v,
                "b_v": b_v,
                "w_pr": w_pr,
                "mask": mask_np,
            }
        )
    return in_maps


def kernel(x, w_qkv, b_qkv, w_proj, b_proj, _spmd_kwargs=None):
    from concourse.bass_utils import run_bass_kernel_spmd

    x = np.asarray(x, dtype=np.float32)
    w_qkv = np.asarray(w_qkv, dtype=np.float32)
    b_qkv = np.asarray(b_qkv, dtype=np.float32)
    w_proj = np.asarray(w_proj, dtype=np.float32)
    b_proj = np.asarray(b_proj, dtype=np.float32)

    nc = _build_module()
    in_maps = _shard_inputs(x, w_qkv, b_qkv, w_proj)
    res = run_bass_kernel_spmd(
        nc, in_maps, list(range(N_CORES)), **(_spmd_kwargs or {})
    )
    out = np.empty((B, S, D), dtype=np.float32)
    for b in range(B):
        acc = np.zeros((S, D), dtype=np.float64)
        for g in range(GROUPS):
            acc += res.results[b * GROUPS + g]["y"]
        out[b] = (acc + b_proj.astype(np.float64)).astype(np.float32)
    if _spmd_kwargs:
        kernel.last_result = res
    return out
